# revision 2
# baseline (speedup 1.0000x reference)
"""Trainium2 Bass kernel for nn_GaussianTrans (axial Gaussian-bias attention).

Math (S=192, C=64, B=4):
  D[q,k] = -(shift*(k-q)^2 + bias)                       (symmetric in q,k)
  Ax = softmax(atten_x[b,r,c,w] + D[c,w], over w)
  Ay = softmax(atten_y[b,c,r,h] + D[r,h], over h)
  out[b,r,c,d] = sum_w Ax[b,r,c,w]*value[b,r,w,d] + sum_h Ay[b,c,r,h]*value[b,h,c,d]

Sharding: 8 cores; core m handles batch b=m//2 and rows rblk = 96*(m%2) ... +96.
Each core computes its full out[b, rblk] block on-device:
  - col part (per c): softmax'd [96r x 192h] @ value[b,:,c,:] -> SBUF accumulator
  - row part (per r): softmax'd [192c x 192w] @ value[b,r]    -> DRAM temp
  - merge: temp + accumulator -> out
The PE contracts over the partition axis, so the exp'd logits need the softmax
axis on partitions; the host pre-transposes both attention tensors so tiles
load in that layout directly (no on-device transposes). Softmax is computed
unnormalized with an inline ones-column in the value operand producing the row
sums from the same matmul; division by the sum is applied to the matmul output.
"""

import sys
import numpy as np

S = 192
C = 64
B = 4
NC = 8
H = S // 2  # rows per core
CB = 4  # c's per column-phase batch

PROFILE_DIR = None  # test harness may set this to capture an NTFF profile

_cache = {}


def _ensure_paths():
    for p in ("/opt/trn_rl_repo", "/root/.axon_site"):
        if p not in sys.path:
            sys.path.insert(0, p)


def _split_waits(nc, mybir):
    """This walrus build allows at most ONE sync-wait per instruction; Tile's
    tail drain can carry several. Move excess waits onto preceding NoOps."""
    for fn in nc.m.functions:
        for blk in fn.blocks:
            out = []
            for inst in list(blk.instructions):
                si = getattr(inst, "sync_info", None)
                if si is not None and si.on_wait is not None and len(si.on_wait) > 1:
                    waits = list(si.on_wait)
                    for k, w in enumerate(waits[:-1]):
                        nop = mybir.InstNoOp(
                            name=f"{inst.name}-wsplit{k}", ins=[], outs=[]
                        )
                        nop.engine = inst.engine
                        nop.sync_info = type(si)(on_update=[], on_wait=[w])
                        out.append(nop)
                    si.on_wait = waits[-1:]
                out.append(inst)
            blk.instructions = out


def _build_nc():
    import concourse.bass as bass
    import concourse.mybir as mybir
    import concourse.tile as tile
    from concourse.vector_clock import ScopedClock

    f32 = mybir.dt.float32
    Exp = mybir.ActivationFunctionType.Exp
    Copy = mybir.ActivationFunctionType.Copy

    class TC(tile.TileContext):
        # The stock tail emits gpsimd dma_reset + sem_clear, which faults the
        # exec unit on this runtime. For a one-shot NEFF the waits + barriers
        # are sufficient; NRT resets semaphore state per launch.
        def _drain_and_barrier(self, tick_clock, wait_clock):
            drain_inst = self.nc.sync.drain()
            wait_clock.add_sem_waits(
                drain_inst.ins, ScopedClock({None: tick_clock.global_clock})
            )
            self.nc.all_engine_barrier()
            self.nc._tile_sem_poison_stack.pop()
            self.nc.all_engine_barrier()

    nc = bass.Bass()
    # axT: atten_x[b, rblk] pre-transposed to [r, w, c]
    ax_d = nc.dram_tensor("axT", (H, S, S), f32, kind="ExternalInput")
    # ayT: atten_y[b, :, rblk, :] pre-transposed to [c, h, r]
    ay_d = nc.dram_tensor("ayT", (S, S, H), f32, kind="ExternalInput")
    val_d = nc.dram_tensor("val", (S, S, C), f32, kind="ExternalInput")
    vrow_d = nc.dram_tensor("vrow", (H, S, C), f32, kind="ExternalInput")
    dfull_d = nc.dram_tensor("dfull", (S, S), f32, kind="ExternalInput")
    dct0_d = nc.dram_tensor("dct0", (128, CB, H), f32, kind="ExternalInput")
    dct1_d = nc.dram_tensor("dct1", (64, CB, H), f32, kind="ExternalInput")
    out_d = nc.dram_tensor("out", (H, S, C), f32, kind="ExternalOutput")
    rtmp_d = nc.dram_tensor("rtmp", (H, S, C), f32, kind="Internal")

    NB = S // CB

    with TC(nc) as tc:
        with tc.tile_pool(name="res", bufs=1) as res:
            vsb0 = res.tile([128, S, C + 1], f32, tag="vsb0")
            vsb1 = res.tile([64, S, C + 1], f32, tag="vsb1")
            d0 = res.tile([128, S], f32, tag="d0")
            d1 = res.tile([64, S], f32, tag="d1")
            dct0 = res.tile([128, CB, H], f32, tag="dct0")
            dct1 = res.tile([64, CB, H], f32, tag="dct1")
            acc = res.tile([H, S, C], f32, tag="acc")

            nc.sync.dma_start(vsb0[:, :, 0:C], val_d[0:128])
            nc.sync.dma_start(vsb1[:, :, 0:C], val_d[128:S])
            nc.vector.memset(vsb0[:, :, C : C + 1], 1.0)
            nc.vector.memset(vsb1[:, :, C : C + 1], 1.0)
            nc.sync.dma_start(d0[:], dfull_d[0:128])
            nc.sync.dma_start(d1[:], dfull_d[128:S])
            nc.sync.dma_start(dct0[:], dct0_d[:])
            nc.sync.dma_start(dct1[:], dct1_d[:])

            # ---- Phase A: column attention, accumulate into acc ----
            with (
                tc.tile_pool(name="ca", bufs=3) as ca,
                tc.tile_pool(name="cops", bufs=8, space="PSUM") as cops,
            ):
                for cb in range(NB):
                    ayt0 = ca.tile([128, CB, H], f32, tag="ayt0")
                    ayt1 = ca.tile([64, CB, H], f32, tag="ayt1")
                    for j in range(CB):
                        c = cb * CB + j
                        nc.sync.dma_start(ayt0[:, j, :], ay_d[c, 0:128, :])
                        nc.sync.dma_start(ayt1[:, j, :], ay_d[c, 128:S, :])
                    nc.vector.tensor_add(ayt0[:], ayt0[:], dct0[:])
                    nc.vector.tensor_add(ayt1[:], ayt1[:], dct1[:])
                    eyt0 = ca.tile([128, CB, H], f32, tag="eyt0")
                    eyt1 = ca.tile([64, CB, H], f32, tag="eyt1")
                    nc.scalar.activation(eyt0[:], ayt0[:], Exp)
                    nc.scalar.activation(eyt1[:], ayt1[:], Exp)
                    for j in range(CB):
                        c = cb * CB + j
                        ocp = cops.tile([H, C + 1], f32, tag="ocp")
                        nc.tensor.matmul(
                            ocp[:], eyt0[:, j, :], vsb0[:, c, :], start=True, stop=False
                        )
                        nc.tensor.matmul(
                            ocp[:], eyt1[:, j, :], vsb1[:, c, :], start=False, stop=True
                        )
                        rec = ca.tile([H, 1], f32, tag="rec")
                        nc.vector.reciprocal(rec[:], ocp[:, C : C + 1])
                        nc.scalar.activation(
                            acc[:, c, :], ocp[:, 0:C], Copy, scale=rec[:]
                        )

            # ---- Phase B: row attention -> rtmp ----
            with (
                tc.tile_pool(name="ra", bufs=4) as ra,
                tc.tile_pool(name="rops", bufs=4, space="PSUM") as rops,
            ):
                for r in range(H):
                    ax0 = ra.tile([128, S], f32, tag="ax0")
                    ax1 = ra.tile([64, S], f32, tag="ax1")
                    nc.sync.dma_start(ax0[:], ax_d[r, 0:128, :])
                    nc.sync.dma_start(ax1[:], ax_d[r, 128:S, :])
                    nc.vector.tensor_add(ax0[:], ax0[:], d0[:])
                    nc.vector.tensor_add(ax1[:], ax1[:], d1[:])
                    vr0 = ra.tile([128, C + 1], f32, tag="vr0")
                    vr1 = ra.tile([64, C + 1], f32, tag="vr1")
                    nc.sync.dma_start(vr0[:, 0:C], vrow_d[r, 0:128, :])
                    nc.sync.dma_start(vr1[:, 0:C], vrow_d[r, 128:S, :])
                    nc.gpsimd.memset(vr0[:, C : C + 1], 1.0)
                    nc.gpsimd.memset(vr1[:, C : C + 1], 1.0)
                    et0 = ra.tile([128, S], f32, tag="et0")
                    et1 = ra.tile([64, S], f32, tag="et1")
                    nc.scalar.activation(et0[:], ax0[:], Exp)
                    nc.scalar.activation(et1[:], ax1[:], Exp)
                    op0 = rops.tile([128, C + 1], f32, tag="op0")
                    op1 = rops.tile([64, C + 1], f32, tag="op1")
                    nc.tensor.matmul(
                        op0[:], et0[:, 0:128], vr0[:], start=True, stop=False
                    )
                    nc.tensor.matmul(
                        op0[:], et1[:, 0:128], vr1[:], start=False, stop=True
                    )
                    nc.tensor.matmul(
                        op1[:], et0[:, 128:S], vr0[:], start=True, stop=False
                    )
                    nc.tensor.matmul(
                        op1[:], et1[:, 128:S], vr1[:], start=False, stop=True
                    )
                    rec0 = ra.tile([128, 1], f32, tag="rec0")
                    rec1 = ra.tile([64, 1], f32, tag="rec1")
                    nc.vector.reciprocal(rec0[:], op0[:, C : C + 1])
                    nc.vector.reciprocal(rec1[:], op1[:, C : C + 1])
                    ob0 = ra.tile([128, C], f32, tag="ob0")
                    ob1 = ra.tile([64, C], f32, tag="ob1")
                    nc.scalar.activation(ob0[:], op0[:, 0:C], Copy, scale=rec0[:])
                    nc.scalar.activation(ob1[:], op1[:, 0:C], Copy, scale=rec1[:])
                    nc.sync.dma_start(rtmp_d[r, 0:128, :], ob0[:])
                    nc.sync.dma_start(rtmp_d[r, 128:S, :], ob1[:])

            # ---- Phase C: merge rtmp + acc -> out ----
            with tc.tile_pool(name="mg", bufs=3) as mg:
                CK = S // 8
                for k in range(8):
                    rt = mg.tile([H, CK, C], f32, tag="rt")
                    nc.sync.dma_start(rt[:], rtmp_d[:, k * CK : (k + 1) * CK, :])
                    nc.vector.tensor_add(
                        rt[:], rt[:], acc[:, k * CK : (k + 1) * CK, :]
                    )
                    nc.sync.dma_start(out_d[:, k * CK : (k + 1) * CK, :], rt[:])

    _split_waits(nc, mybir)
    return nc


def _get_runner():
    if "runner" in _cache:
        return _cache["runner"]
    _ensure_paths()
    import jax
    import concourse.mybir as mybir
    from jax.sharding import Mesh, PartitionSpec
    from jax.experimental.shard_map import shard_map
    from concourse import bass2jax
    from concourse.bass2jax import _bass_exec_p, install_neuronx_cc_hook

    nc = _build_nc()
    install_neuronx_cc_hook()

    partition_name = nc.partition_id_tensor.name if nc.partition_id_tensor else None
    in_names, out_names, out_avals, zero_shapes = [], [], [], []
    for alloc in nc.m.functions[0].allocations:
        if not isinstance(alloc, mybir.MemoryLocationSet):
            continue
        name = alloc.memorylocations[0].name
        if alloc.kind == "ExternalInput":
            if name != partition_name:
                in_names.append(name)
        elif alloc.kind == "ExternalOutput":
            shape = tuple(alloc.tensor_shape)
            dtype = mybir.dt.np(alloc.dtype)
            out_names.append(name)
            out_avals.append(jax.core.ShapedArray(shape, dtype))
            zero_shapes.append((shape, dtype))
    n_params = len(in_names)
    n_outs = len(out_names)
    all_names = in_names + out_names
    if partition_name is not None:
        all_names = all_names + [partition_name]
    donate = tuple(range(n_params, n_params + n_outs))

    def _body(*args):
        operands = list(args)
        if partition_name is not None:
            operands.append(bass2jax.partition_id_tensor())
        outs = _bass_exec_p.bind(
            *operands,
            out_avals=tuple(out_avals),
            in_names=tuple(all_names),
            out_names=tuple(out_names),
            lowering_input_output_aliases=(),
            sim_require_finite=True,
            sim_require_nnan=True,
            nc=nc,
        )
        return tuple(outs)

    devices = jax.devices()[:NC]
    mesh = Mesh(np.asarray(devices), ("core",))
    in_specs = (PartitionSpec("core"),) * (n_params + n_outs)
    out_specs = (PartitionSpec("core"),) * n_outs
    sharded = jax.jit(
        shard_map(
            _body, mesh=mesh, in_specs=in_specs, out_specs=out_specs, check_rep=False
        ),
        donate_argnums=donate,
        keep_unused=True,
    )

    def run(in_maps):
        concat_in = [
            np.concatenate([np.asarray(in_maps[c][k]) for c in range(NC)], axis=0)
            for k in in_names
        ]
        concat_zeros = [
            np.zeros((NC * sh[0], *sh[1:]), dt) for (sh, dt) in zero_shapes
        ]
        out_arrs = sharded(*concat_in, *concat_zeros)
        return [
            {
                name: np.asarray(out_arrs[i]).reshape(NC, *out_avals[i].shape)[c]
                for i, name in enumerate(out_names)
            }
            for c in range(NC)
        ]

    _cache["runner"] = run
    return run


def kernel(x, atten_x_full, atten_y_full, value_full, shift, bias):
    _ensure_paths()
    run = _get_runner()

    atten_x_full = np.asarray(atten_x_full, np.float32)
    atten_y_full = np.asarray(atten_y_full, np.float32)
    value_full = np.asarray(value_full, np.float32)
    shift = np.asarray(shift, np.float32)
    bias = np.asarray(bias, np.float32)

    idx = np.arange(S, dtype=np.float32)
    dist2 = (idx[None, :] - idx[:, None]) ** 2
    D = -(shift[0] * dist2 + bias[0])

    in_maps = []
    for m in range(NC):
        b, half = m // 2, m % 2
        rblk = slice(half * H, (half + 1) * H)
        in_maps.append(
            {
                # [r, w, c]: per-r tiles load with w (contraction) on partitions
                "axT": np.ascontiguousarray(
                    atten_x_full[b, rblk].transpose(0, 2, 1)
                ),
                # [c, h, r]: per-c tiles load with h (contraction) on partitions
                "ayT": np.ascontiguousarray(
                    atten_y_full[b, :, rblk, :].transpose(0, 2, 1)
                ),
                "val": value_full[b],
                "vrow": np.ascontiguousarray(value_full[b, rblk]),
                "dfull": D,
                "dct0": np.ascontiguousarray(
                    np.broadcast_to(D[0:128, rblk][:, None, :], (128, CB, H))
                ),
                "dct1": np.ascontiguousarray(
                    np.broadcast_to(D[128:S, rblk][:, None, :], (64, CB, H))
                ),
            }
        )

    if PROFILE_DIR is not None:
        from trn_agent_boot.trn_boot import _ntff_profile_via_ctypes

        hook = _ntff_profile_via_ctypes("/opt/axon/libaxon_pjrt.so")
        with hook(PROFILE_DIR, [0]):
            results = run(in_maps)
    else:
        results = run(in_maps)

    out = np.empty((B, S, S, C), np.float32)
    for m in range(NC):
        b, half = m // 2, m % 2
        out[b, half * H : (half + 1) * H] = results[m]["out"]
    return out
